# revision 17
# baseline (speedup 1.0000x reference)
"""Trainium2 Bass kernel for nn_ContextPromptGenerator.

Math restructure (as baseline): pooled bins are masked segment sums over
tokens, so the 0/1-mask matmul runs FIRST on [T, 4096] packed rows, then
the 4096->1024 down-projection runs on pooled rows only.

v3 over the 201us baseline:
- TP-pair: cores are paired {2g, 2g+1}; a pair shares its 4 samples
  (128 bins).  Each member computes ALL 128 pair bins but only HALF of
  the D dimension (Wd[:, 512m:+512], 4MB) and HALF of the H output
  (Wu[:, 2048m:+2048], 4MB) -- weight DMA drops 16MB -> 8MB/core.  Bin
  sums are exchanged via 2-rank AllGathers (2 x 256KB xsT chunks,
  1 x 128KB siluT) on replica groups [[0,1],[2,3],[4,5],[6,7]].
- ctx path is pair-local: each core sums ONLY its own 2 samples'
  context embeddings (no cross-core reduction exists, so the 42us
  8-rank AllReduce of v2 is gone); a 16KB pair-AllGather supplies the
  partner's sums; ctx_d[4, 512-half] comes from the fp8 Wc half.
- eh/cmask/Wc-half are fp8e4 (emb x64, Wc x16; compensated in the fp32
  aug coefficients /1024).  Verified: error unchanged (4.1e-4).
- Phase D runs full-array: the two xs slabs are interleaved on-chip
  (DVE strided copy) into [128, hc x 128bins] so each D matmul carries
  128 weight columns.
- DMA: one sync-queue FIFO in priority order (masks/eh -> x -> Wd ->
  Wc -> Wu); x streams through a 3-buffer pool in 1MB pieces; weights
  are 0.5-4MB pieces consumed slice-wise by D/U as they land.
- Output is written fp16 (host upcasts); halves the out DMA.

Sharding: data-parallel over samples for the bin sums (2 per core,
paired to minimize roundup128(max seq pair)); TP-pair for D/U and ctx.
"""

import numpy as np
import ml_dtypes
from contextlib import ExitStack

import concourse.bass as bass
import concourse.mybir as mybir
import concourse.tile as tile
from concourse import bacc
from concourse.masks import make_identity
from concourse.bass_utils import run_bass_kernel_spmd

F32 = mybir.dt.float32
F16 = mybir.dt.float16
F8 = mybir.dt.float8e4
F8NP = ml_dtypes.float8_e4m3

B, S, C, H, D, V, P = 16, 2048, 512, 4096, 1024, 32000, 32
NC = 8          # cores
SPC = 2         # samples per core
M = SPC * P     # 64 own bins per core
MB2 = 2 * M     # 128 bins per core-pair
HT = H // 128   # 32 h-tiles
DT = D // 128   # 8 d-tiles
DH = D // 2     # 512: d half (per pair member)
HH = H // 2     # 2048: h half (per pair member)
RG2 = [[0, 1], [2, 3], [4, 5], [6, 7]]
EH_SCALE = 64.0      # emb fp8 pre-scale
WC_SCALE = 16.0      # Wc fp8 pre-scale
CTX_SCALE = EH_SCALE * WC_SCALE

_cache = {}


def _build(T, Tc):
    """Per-core SPMD Bass program.  T = packed hidden rows per core,
    Tc = packed own-2-sample context rows per core (both mult. of 128)."""
    nc = bacc.Bacc(None, target_bir_lowering=False, num_devices=NC)

    KT = T // 128
    KC = Tc // 128

    # ---- dram I/O ----
    xh_d = nc.dram_tensor("xh", [128, 2 * KT * HH], F16, kind="ExternalInput")
    mx_d = nc.dram_tensor("mxr", [128, KT * M], F16, kind="ExternalInput")
    eh_d = nc.dram_tensor("eh8", [128, KC * H], F8, kind="ExternalInput")
    cm_d = nc.dram_tensor("cm8", [128, KC * SPC], F8, kind="ExternalInput")
    wch_d = nc.dram_tensor("wch8", [128, HT * DH], F8, kind="ExternalInput")
    wdh_d = nc.dram_tensor("wdh", [128, HT * DH], F16, kind="ExternalInput")
    wuh_d = nc.dram_tensor("wuh", [128, DT * HH], F16, kind="ExternalInput")
    bd_d = nc.dram_tensor("bdh", [1, DH], F32, kind="ExternalInput")
    bc_d = nc.dram_tensor("bch", [1, DH], F32, kind="ExternalInput")
    bu_d = nc.dram_tensor("buh", [1, HH], F16, kind="ExternalInput")
    aug_d = nc.dram_tensor("aug", [6, MB2], F32, kind="ExternalInput")
    sinv_d = nc.dram_tensor("sinv", [MB2, 1], F32, kind="ExternalInput")
    out_d = nc.dram_tensor("out", [MB2, HH], F16, kind="ExternalOutput")
    # collective bounce buffers (internal DRAM)
    csp_d = nc.dram_tensor("csp", [128, HT * SPC], F16)
    csg_d = nc.dram_tensor("csg", [256, HT * SPC], F16)
    xsp_d = [nc.dram_tensor(f"xsp{g}", [128, 16 * M], F16) for g in range(2)]
    xsg_d = [nc.dram_tensor(f"xsg{g}", [256, 16 * M], F16) for g in range(2)]
    stp_d = nc.dram_tensor("stp", [128, 4 * MB2], F16)
    stg_d = nc.dram_tensor("stg", [256, 4 * MB2], F16)

    with tile.TileContext(nc) as tc, ExitStack() as ctx:
        const = ctx.enter_context(tc.tile_pool(name="const", bufs=1))
        keep = ctx.enter_context(tc.tile_pool(name="keep", bufs=1))
        xpool = ctx.enter_context(tc.tile_pool(name="xpool", bufs=3))
        opool = ctx.enter_context(tc.tile_pool(name="opool", bufs=1))

        ident16 = const.tile([128, 128], F16)
        idtmp = const.tile([128, 128], F32)
        make_identity(nc, idtmp)
        nc.vector.tensor_copy(ident16, idtmp)
        ones1 = const.tile([1, 128], F16)
        nc.vector.memset(ones1, 1.0)

        # ---- sync-queue input DMAs, emitted in priority order ----
        cm8_sb = keep.tile([128, KC * SPC], F8)
        nc.sync.dma_start(out=cm8_sb, in_=cm_d[:, :])
        eh_sb = keep.tile([128, KC * H], F8)
        ehmid = (KC // 2) * H
        if ehmid > 0:
            nc.sync.dma_start(out=eh_sb[:, 0:ehmid], in_=eh_d[:, 0:ehmid])
        nc.sync.dma_start(out=eh_sb[:, ehmid:], in_=eh_d[:, ehmid:])
        mxr_sb = keep.tile([128, KT * M], F16)
        nc.sync.dma_start(out=mxr_sb, in_=mx_d[:, :])
        aug_sb = keep.tile([6, MB2], F32)
        nc.sync.dma_start(out=aug_sb, in_=aug_d[:, :])
        sinv_sb = keep.tile([MB2, 1], F32)
        nc.sync.dma_start(out=sinv_sb, in_=sinv_d[:, :])
        augr_sb = keep.tile([6, DH], F32)
        nc.sync.dma_start(out=augr_sb[4:5, :], in_=bd_d[:, :])
        nc.sync.dma_start(out=augr_sb[5:6, :], in_=bc_d[:, :])

        # weight tiles (DMAs emitted AFTER phase X: x streams first)
        wdh_sb = keep.tile([128, HT * DH], F16)
        wch_sb = keep.tile([128, HT * DH], F8)
        wuh_sb = keep.tile([128, DT * HH], F16)
        buh_sb = keep.tile([1, HH], F16)

        cs16 = keep.tile([128, HT * SPC], F16)
        cs4 = keep.tile([128, HT * 4], F8)
        xs_c = keep.tile([128, 2 * 16 * M], F16)
        xsA = [keep.tile([128, 16 * M], F16, name=f"xsA{g}") for g in range(2)]
        xsB = [keep.tile([128, 16 * M], F16, name=f"xsB{g}") for g in range(2)]
        xsAB = [keep.tile([128, 16 * 128], F16, name=f"xsAB{g}")
                for g in range(2)]
        silu_sb = keep.tile([MB2, DH], F16)
        st_own = keep.tile([128, 4 * MB2], F16)
        stA = keep.tile([128, 4 * MB2], F16)
        stB = keep.tile([128, 4 * MB2], F16)

        with tc.tile_pool(name="psA", bufs=1, space="PSUM") as psA, \
                tc.tile_pool(name="psX", bufs=1, space="PSUM") as psX, \
                tc.tile_pool(name="psB", bufs=1, space="PSUM") as psB, \
                tc.tile_pool(name="psD", bufs=1, space="PSUM") as psD:
            # weight DMAs FIRST (wch+wd ahead of x, wu behind it): the
            # exposed post-X collectives then run against an idle DMA and
            # D starts at full PE speed the moment its AllGather lands.
            nc.sync.dma_start(out=wch_sb, in_=wch_d[:, :])
            for q in range(2):
                c0, c1 = q * 16 * DH, (q + 1) * 16 * DH
                nc.sync.dma_start(out=wdh_sb[:, c0:c1], in_=wdh_d[:, c0:c1])

            # ---- phase A: own-2-sample ctx_sumT[h, s] (fills the PE
            # window before the first x piece lands) ----
            ps_ctx = psA.tile([128, HT * SPC], F32)
            for kc in range(KC):
                for hc in range(HT):
                    nc.tensor.matmul(
                        ps_ctx[:, SPC * hc:SPC * (hc + 1)],
                        eh_sb[:, kc * H + 128 * hc:kc * H + 128 * (hc + 1)],
                        cm8_sb[:, SPC * kc:SPC * (kc + 1)],
                        start=(kc == 0 and hc == 0),
                        stop=(kc == KC - 1),
                    )
            nc.vector.tensor_copy(cs16, ps_ctx)
            # cs pair-exchange: first on the TOPSP chain, hidden under x
            nc.gpsimd.dma_start(out=csp_d[:, :], in_=cs16)
            nc.gpsimd.collective_compute(
                "AllGather", mybir.AluOpType.bypass, replica_groups=RG2,
                ins=[csp_d[:, :].opt()], outs=[csg_d[:, :].opt()])
            # gathered cs -> interleaved [A0 A1 B0 B1] per hc, f16->f8
            cv = cs4.rearrange("p (hc w) -> p hc w", w=4)
            nc.gpsimd.dma_start(out=cv[:, :, 0:2], in_=csg_d[0:128, :])
            nc.gpsimd.dma_start(out=cv[:, :, 2:4], in_=csg_d[128:256, :])

            # ---- phase X: xsumT[h, j] in 2 h-chunks of 16 h-tiles ----
            # x streams through xpool, 4 k-tiles (2MB) per buffer
            ps_xs = psX.tile([128, HT * M], F32)  # 4 banks
            KB = (KT + 3) // 4
            for g in range(2):
                for kb in range(KB):
                    nk = min(4, KT - 4 * kb)
                    xt = xpool.tile([128, 4 * HH], F16, tag="xkb")
                    nc.sync.dma_start(
                        out=xt[:, 0:nk * HH],
                        in_=xh_d[:, g * KT * HH + 4 * kb * HH:
                                 g * KT * HH + (4 * kb + nk) * HH])
                    for kk in range(nk):
                        k = 4 * kb + kk
                        for hcl in range(16):
                            nc.tensor.matmul(
                                ps_xs[:, (16 * g + hcl) * M:
                                      (16 * g + hcl + 1) * M],
                                xt[:, kk * HH + 128 * hcl:
                                   kk * HH + 128 * (hcl + 1)],
                                mxr_sb[:, M * k:M * (k + 1)],
                                start=(k == 0 and hcl % 8 == 0),
                                stop=(k == KT - 1),
                            )
                cg0, cg1 = g * 16 * M, (g + 1) * 16 * M
                nc.vector.tensor_copy(xs_c[:, cg0:cg1], ps_xs[:, cg0:cg1])
                nc.gpsimd.dma_start(out=xsp_d[g][:, :], in_=xs_c[:, cg0:cg1])
                nc.gpsimd.collective_compute(
                    "AllGather", mybir.AluOpType.bypass, replica_groups=RG2,
                    ins=[xsp_d[g][:, :].opt()], outs=[xsg_d[g][:, :].opt()])
                nc.gpsimd.dma_start(out=xsA[g], in_=xsg_d[g][0:128, :])
                nc.gpsimd.dma_start(out=xsB[g], in_=xsg_d[g][128:256, :])
                # interleave the two slabs: [128, hc x (A 64 | B 64)];
                # on gpsimd so it queues right behind its own readbacks
                # (a DVE-queue copy here would head-of-line-block the
                # chunk-1 xs copy behind chunk-0's AllGather)
                v = xsAB[g].rearrange("p (hc w) -> p hc w", w=128)
                nc.gpsimd.tensor_copy(
                    v[:, :, 0:64],
                    xsA[g].rearrange("p (hc w) -> p hc w", w=64))
                nc.gpsimd.tensor_copy(
                    v[:, :, 64:128],
                    xsB[g].rearrange("p (hc w) -> p hc w", w=64))

            # wu stream behind x; U consumes it ~15us after it lands
            for q in range(2):
                c0, c1 = q * 4 * HH, (q + 1) * 4 * HH
                nc.sync.dma_start(out=wuh_sb[:, c0:c1], in_=wuh_d[:, c0:c1])
            nc.sync.dma_start(out=buh_sb, in_=bu_d[:, :])

            # ---- phase D (chunk 0), B, D (chunk 1): weights resident ----
            ps_d = psD.tile([128, DH], F32)   # 1 bank
            ps_cd = psB.tile([4, DH], F32)    # 1 bank
            for hc in range(16):
                nc.tensor.matmul(
                    ps_d,
                    xsAB[0][:, 128 * hc:128 * (hc + 1)],
                    wdh_sb[:, DH * hc:DH * (hc + 1)],
                    start=(hc == 0), stop=False)
            for hc in range(HT):
                nc.tensor.matmul(
                    ps_cd,
                    cs4[:, 4 * hc:4 * (hc + 1)],
                    wch_sb[:, DH * hc:DH * (hc + 1)],
                    start=(hc == 0), stop=(hc == HT - 1))
            for hcl in range(16):
                hc = 16 + hcl
                nc.tensor.matmul(
                    ps_d,
                    xsAB[1][:, 128 * hcl:128 * (hcl + 1)],
                    wdh_sb[:, DH * hc:DH * (hc + 1)],
                    start=False, stop=False)
            nc.vector.tensor_copy(augr_sb[0:4, :], ps_cd)

            # aug term closes the D accumulation, then scaled silu
            nc.tensor.matmul(ps_d, aug_sb, augr_sb, start=False, stop=True)
            nc.scalar.activation(
                silu_sb, ps_d,
                mybir.ActivationFunctionType.Silu, scale=sinv_sb)

        # ---- phase E: siluT [d-half, 128 bins] ----
        with tc.tile_pool(name="psE", bufs=2, space="PSUM") as psE:
            for dc in range(4):
                pst = psE.tile([128, 128], F16, tag="silutr")
                nc.tensor.transpose(
                    pst, silu_sb[:, 128 * dc:128 * (dc + 1)], ident16)
                nc.vector.tensor_copy(
                    st_own[:, 128 * dc:128 * (dc + 1)], pst)
        nc.gpsimd.dma_start(out=stp_d[:, :], in_=st_own)
        nc.gpsimd.collective_compute(
            "AllGather", mybir.AluOpType.bypass, replica_groups=RG2,
            ins=[stp_d[:, :].opt()], outs=[stg_d[:, :].opt()])
        nc.gpsimd.dma_start(out=stA, in_=stg_d[0:128, :])
        nc.gpsimd.dma_start(out=stB, in_=stg_d[128:256, :])

        # ---- phase U: out[pair bins, h-half] ----
        with tc.tile_pool(name="psU", bufs=1, space="PSUM") as psU:
            ps_u = psU.tile([MB2, HH], F32)  # 4 banks
            for dk in range(DT):
                sl = stA if dk < 4 else stB
                lhsT = sl[:, 128 * (dk % 4):128 * (dk % 4 + 1)]
                for nb in range(4):
                    nc.tensor.matmul(
                        ps_u[:, 512 * nb:512 * (nb + 1)],
                        lhsT,
                        wuh_sb[:, HH * dk + 512 * nb:HH * dk + 512 * (nb + 1)],
                        start=(dk == 0), stop=False)
            for nb in range(4):
                nc.tensor.matmul(
                    ps_u[:, 512 * nb:512 * (nb + 1)],
                    ones1,
                    buh_sb[:, 512 * nb:512 * (nb + 1)],
                    start=False, stop=True)
            ot = opool.tile([MB2, HH], F16)
            for nb in range(4):
                nc.vector.tensor_copy(
                    ot[:, 512 * nb:512 * (nb + 1)],
                    ps_u[:, 512 * nb:512 * (nb + 1)])
                if nb % 2 == 1:
                    nc.sync.dma_start(
                        out=out_d[:, 512 * (nb - 1):512 * (nb + 1)],
                        in_=ot[:, 512 * (nb - 1):512 * (nb + 1)])

    nc.finalize()
    return nc


def _roundup(v, m):
    return max(m, ((int(v) + m - 1) // m) * m)


def _pm(a, kt):
    """Reorder [kt*128, cols] row-major -> partition-major [128, kt*cols]."""
    n, cols = a.shape
    assert n == kt * 128
    return np.ascontiguousarray(
        a.reshape(kt, 128, cols).transpose(1, 0, 2).reshape(128, kt * cols))


def _f8(a):
    return np.ascontiguousarray(np.asarray(a, dtype=np.float32)).astype(F8NP)


def _pair_samples(seq):
    """Pair the 16 samples 2-per-core minimizing roundup128(max pair seq).
    Greedy sort-and-reflect, then 2-opt passes."""
    order = np.argsort(-seq, kind="stable")
    pairs = [[int(order[i]), int(order[2 * NC - 1 - i])] for i in range(NC)]

    def cost(ps):
        return (_roundup(max(seq[a] + seq[b] for a, b in ps), 128),
                max(seq[a] + seq[b] for a, b in ps))

    best = cost(pairs)
    improved = True
    while improved:
        improved = False
        for i in range(NC):
            for j in range(i + 1, NC):
                for swap in ((1, 1), (1, 0), (0, 1)):
                    cand = [list(p) for p in pairs]
                    cand[i][swap[0]], cand[j][swap[1]] = \
                        cand[j][swap[1]], cand[i][swap[0]]
                    c = cost(cand)
                    if c < best:
                        best, pairs, improved = c, cand, True
    return [(a, b) for a, b in pairs]


def kernel(**inputs):
    ids = np.asarray(inputs["context_ids"]).astype(np.int64)
    x = np.asarray(inputs["hidden_states"], dtype=np.float32)
    seq = np.asarray(inputs["seq_lengths"]).astype(np.int64)
    clen = np.asarray(inputs["context_lengths"]).astype(np.int64)
    emb = np.asarray(inputs["embed_table"], dtype=np.float32)
    Wc = np.ascontiguousarray(inputs["Wc"], dtype=np.float32)
    bc = np.asarray(inputs["bc"], dtype=np.float32)
    Wd = np.ascontiguousarray(inputs["Wd"], dtype=np.float32)
    bd = np.asarray(inputs["bd"], dtype=np.float32)
    Wu = np.ascontiguousarray(inputs["Wu"], dtype=np.float32)
    bu = np.asarray(inputs["bu"], dtype=np.float32)

    assert x.shape == (B, S, H) and ids.shape == (B, C)

    # per-sample bin geometry
    L = seq + 1
    jj = np.arange(P, dtype=np.int64)
    start = (jj[None, :] * L[:, None]) // P            # [B,P]
    end = ((jj[None, :] + 1) * L[:, None] + P - 1) // P
    Sj = (end - start).astype(np.float32)
    lo = np.maximum(start - 1, 0)
    hi = end - 1
    cnt = (hi - lo).astype(np.float32)
    ind = (start == 0).astype(np.float32)

    pairs = _pair_samples(seq)
    T = _roundup(max(seq[a] + seq[b] for a, b in pairs), 128)
    KT = T // 128

    clen_c = np.maximum(clen, 1)
    Tc = _roundup(max(clen_c[a] + clen_c[b] for a, b in pairs), 128)
    KC = Tc // 128

    key = (T, Tc)
    if key not in _cache:
        _cache[key] = _build(T, Tc)
    nc = _cache[key]

    # host-side weight layouts
    emb8 = _f8(emb * EH_SCALE)
    wd16 = Wd.astype(np.float16)
    wu16 = Wu.astype(np.float16)
    wdh = [_pm(np.ascontiguousarray(wd16[:, DH * m:DH * (m + 1)]), HT)
           for m in range(2)]
    wch = [_pm(_f8(Wc[:, DH * m:DH * (m + 1)] * WC_SCALE), HT)
           for m in range(2)]
    wuh = [np.ascontiguousarray(
        wu16[:, HH * m:HH * (m + 1)].reshape(DT, 128, HH)
        .transpose(1, 0, 2).reshape(128, DT * HH)) for m in range(2)]
    bdh = [bd[DH * m:DH * (m + 1)].reshape(1, DH) for m in range(2)]
    bch = [bc[DH * m:DH * (m + 1)].reshape(1, DH) for m in range(2)]
    buh = [bu[HH * m:HH * (m + 1)].astype(np.float16).reshape(1, HH)
           for m in range(2)]

    in_maps = []
    for c, (a, b) in enumerate(pairs):
        g, m = c // 2, c % 2
        sa, sb = int(seq[a]), int(seq[b])
        xp = np.zeros((T, H), np.float16)
        xp[:sa] = x[a, :sa]
        xp[sa:sa + sb] = x[b, :sb]
        # chunk-major reorder: [128, g(2) x KT x 2048]
        xr = np.ascontiguousarray(
            xp.reshape(KT, 128, 2, HH).transpose(1, 2, 0, 3)
            .reshape(128, 2 * KT * HH))
        t = np.arange(T, dtype=np.int64)[:, None]
        mx = np.zeros((T, M), np.float16)
        mx[:, :P] = ((t >= lo[a][None, :]) & (t < hi[a][None, :]))
        mx[:, P:] = ((t - sa >= lo[b][None, :]) & (t - sa < hi[b][None, :])
                     & (t >= sa))
        # own 2 samples' context rows + 2-col one-hot
        ca, cb = int(clen_c[a]), int(clen_c[b])
        ep = np.zeros((Tc, H), F8NP)
        ep[:ca] = emb8[ids[a, :ca]]
        ep[ca:ca + cb] = emb8[ids[b, :cb]]
        cm = np.zeros((Tc, SPC), np.float32)
        cm[:ca, 0] = 1.0
        cm[ca:ca + cb, 1] = 1.0
        # group bin order: [pair0 sample a bins, pair0 b, pair1 a, pair1 b]
        gs = [pairs[2 * g][0], pairs[2 * g][1],
              pairs[2 * g + 1][0], pairs[2 * g + 1][1]]
        # augr rows: 0..3 = ctx_d of gs[0..3]; 4 = bd; 5 = bc
        aug = np.zeros((6, MB2), np.float32)
        sinv = np.zeros((MB2, 1), np.float32)
        for i, s in enumerate(gs):
            sl = slice(P * i, P * (i + 1))
            aug[i, sl] = ind[s] / (clen_c[s] * CTX_SCALE)
            aug[4, sl] = cnt[s]
            aug[5, sl] = ind[s]
            sinv[sl, 0] = 1.0 / Sj[s]
        in_maps.append({
            "xh": xr, "mxr": _pm(mx, KT),
            "eh8": _pm(ep, KC), "cm8": _pm(cm.astype(F8NP), KC),
            "wch8": wch[m], "wdh": wdh[m], "wuh": wuh[m],
            "bdh": bdh[m], "bch": bch[m], "buh": buh[m],
            "aug": aug, "sinv": sinv,
        })

    res = run_bass_kernel_spmd(nc, in_maps, core_ids=list(range(NC)))
    _cache["last_result"] = res

    out = np.empty((B, P, H), np.float32)
    for c in range(NC):
        g, m = c // 2, c % 2
        o = np.asarray(res.results[c]["out"], dtype=np.float32)
        gs = [pairs[2 * g][0], pairs[2 * g][1],
              pairs[2 * g + 1][0], pairs[2 * g + 1][1]]
        for i, s in enumerate(gs):
            out[s, :, HH * m:HH * (m + 1)] = o[P * i:P * (i + 1), :]
    return out


# revision 21
# speedup vs baseline: 1.0931x; 1.0931x over previous
"""Trainium2 Bass kernel for nn_ContextPromptGenerator.

Math restructure (as baseline): pooled bins are masked segment sums over
tokens, so the 0/1-mask matmul runs FIRST on [T, 4096] packed rows, then
the 4096->1024 down-projection runs on pooled rows only.

v3 over the 201us baseline:
- TP-pair: cores are paired {2g, 2g+1}; a pair shares its 4 samples
  (128 bins).  Each member computes ALL 128 pair bins but only HALF of
  the D dimension (Wd[:, 512m:+512], 4MB) and HALF of the H output
  (Wu[:, 2048m:+2048], 4MB) -- weight DMA drops 16MB -> 8MB/core.  Bin
  sums are exchanged via 2-rank AllGathers (2 x 256KB xsT chunks,
  1 x 128KB siluT) on replica groups [[0,1],[2,3],[4,5],[6,7]].
- ctx path is pair-local: each core sums ONLY its own 2 samples'
  context embeddings (no cross-core reduction exists, so the 42us
  8-rank AllReduce of v2 is gone); a 16KB pair-AllGather supplies the
  partner's sums; ctx_d[4, 512-half] comes from the fp8 Wc half.
- eh/cmask/Wc-half are fp8e4 (emb x64, Wc x16; compensated in the fp32
  aug coefficients /1024).  Verified: error unchanged (4.1e-4).
- Phase D runs full-array: the two xs slabs are interleaved on-chip
  (DVE strided copy) into [128, hc x 128bins] so each D matmul carries
  128 weight columns.
- DMA: one sync-queue FIFO in priority order (masks/eh -> x -> Wd ->
  Wc -> Wu); x streams through a 3-buffer pool in 1MB pieces; weights
  are 0.5-4MB pieces consumed slice-wise by D/U as they land.
- Output is written fp16 (host upcasts); halves the out DMA.

Sharding: data-parallel over samples for the bin sums (2 per core,
paired to minimize roundup128(max seq pair)); TP-pair for D/U and ctx.
"""

import numpy as np
import ml_dtypes
from contextlib import ExitStack

import concourse.bass as bass
import concourse.mybir as mybir
import concourse.tile as tile
from concourse import bacc
from concourse.masks import make_identity
from concourse.bass_utils import run_bass_kernel_spmd

F32 = mybir.dt.float32
F16 = mybir.dt.float16
F8 = mybir.dt.float8e4
F8NP = ml_dtypes.float8_e4m3

B, S, C, H, D, V, P = 16, 2048, 512, 4096, 1024, 32000, 32
NC = 8          # cores
SPC = 2         # samples per core
M = SPC * P     # 64 own bins per core
MB2 = 2 * M     # 128 bins per core-pair
HT = H // 128   # 32 h-tiles
DT = D // 128   # 8 d-tiles
DH = D // 2     # 512: d half (per pair member)
HH = H // 2     # 2048: h half (per pair member)
RG2 = [[0, 1], [2, 3], [4, 5], [6, 7]]
EH_SCALE = 64.0      # emb fp8 pre-scale
WC_SCALE = 16.0      # Wc fp8 pre-scale
CTX_SCALE = EH_SCALE * WC_SCALE

_cache = {}


def _build(T, Tc):
    """Per-core SPMD Bass program.  T = packed hidden rows per core,
    Tc = packed own-2-sample context rows per core (both mult. of 128)."""
    nc = bacc.Bacc(None, target_bir_lowering=False, num_devices=NC)

    KT = T // 128
    KC = Tc // 128

    # ---- dram I/O ----
    xh_d = nc.dram_tensor("xh", [128, KT * H], F16, kind="ExternalInput")
    mx_d = nc.dram_tensor("mxr", [128, KT * M], F16, kind="ExternalInput")
    eh_d = nc.dram_tensor("eh8", [128, KC * H], F8, kind="ExternalInput")
    cm_d = nc.dram_tensor("cm8", [128, KC * SPC], F8, kind="ExternalInput")
    wch_d = nc.dram_tensor("wch8", [128, HT * DH], F8, kind="ExternalInput")
    wdh_d = nc.dram_tensor("wdh", [128, HT * DH], F16, kind="ExternalInput")
    wuh_d = nc.dram_tensor("wuh", [128, DT * HH], F16, kind="ExternalInput")
    bd_d = nc.dram_tensor("bdh", [1, DH], F32, kind="ExternalInput")
    bc_d = nc.dram_tensor("bch", [1, DH], F32, kind="ExternalInput")
    bu_d = nc.dram_tensor("buh", [1, HH], F16, kind="ExternalInput")
    aug_d = nc.dram_tensor("aug", [6, MB2], F32, kind="ExternalInput")
    sinv_d = nc.dram_tensor("sinv", [MB2, 1], F32, kind="ExternalInput")
    out_d = nc.dram_tensor("out", [MB2, HH], F16, kind="ExternalOutput")
    # collective bounce buffers (internal DRAM)
    csp_d = nc.dram_tensor("csp", [128, HT * SPC], F16)
    csg_d = nc.dram_tensor("csg", [256, HT * SPC], F16)
    CH = (24, 8)  # h-tiles per X chunk (bank-aligned: 3 banks + 1)
    xsp_d = [nc.dram_tensor(f"xsp{g}", [128, CH[g] * M], F16)
             for g in range(2)]
    xsg_d = [nc.dram_tensor(f"xsg{g}", [256, CH[g] * M], F16)
             for g in range(2)]
    stp_d = nc.dram_tensor("stp", [128, 4 * MB2], F16)
    stg_d = nc.dram_tensor("stg", [256, 4 * MB2], F16)

    with tile.TileContext(nc) as tc, ExitStack() as ctx:
        const = ctx.enter_context(tc.tile_pool(name="const", bufs=1))
        keep = ctx.enter_context(tc.tile_pool(name="keep", bufs=1))
        xpool = ctx.enter_context(tc.tile_pool(name="xpool", bufs=3))
        opool = ctx.enter_context(tc.tile_pool(name="opool", bufs=1))

        ident16 = const.tile([128, 128], F16)
        idtmp = const.tile([128, 128], F32)
        make_identity(nc, idtmp)
        nc.vector.tensor_copy(ident16, idtmp)
        ones1 = const.tile([1, 128], F16)
        nc.vector.memset(ones1, 1.0)

        # ---- sync-queue input DMAs, emitted in priority order ----
        cm8_sb = keep.tile([128, KC * SPC], F8)
        nc.sync.dma_start(out=cm8_sb, in_=cm_d[:, :])
        eh_sb = keep.tile([128, KC * H], F8)
        ehmid = (KC // 2) * H
        if ehmid > 0:
            nc.sync.dma_start(out=eh_sb[:, 0:ehmid], in_=eh_d[:, 0:ehmid])
        nc.sync.dma_start(out=eh_sb[:, ehmid:], in_=eh_d[:, ehmid:])
        mxr_sb = keep.tile([128, KT * M], F16)
        nc.sync.dma_start(out=mxr_sb, in_=mx_d[:, :])
        aug_sb = keep.tile([6, MB2], F32)
        nc.sync.dma_start(out=aug_sb, in_=aug_d[:, :])
        sinv_sb = keep.tile([MB2, 1], F32)
        nc.sync.dma_start(out=sinv_sb, in_=sinv_d[:, :])
        augr_sb = keep.tile([6, DH], F32)
        nc.sync.dma_start(out=augr_sb[4:5, :], in_=bd_d[:, :])
        nc.sync.dma_start(out=augr_sb[5:6, :], in_=bc_d[:, :])

        # weight tiles (DMAs emitted AFTER phase X: x streams first)
        wdh_sb = keep.tile([128, HT * DH], F16)
        wch_sb = keep.tile([128, HT * DH], F8)
        wuh_sb = keep.tile([128, DT * HH], F16)
        buh_sb = keep.tile([1, HH], F16)

        cs16 = keep.tile([128, HT * SPC], F16)
        cs4 = keep.tile([128, HT * 4], F8)
        xs_c = keep.tile([128, HT * M], F16)
        xsA = [keep.tile([128, CH[g] * M], F16, name=f"xsA{g}")
               for g in range(2)]
        xsB = [keep.tile([128, CH[g] * M], F16, name=f"xsB{g}")
               for g in range(2)]
        xsAB = [keep.tile([128, CH[g] * 128], F16, name=f"xsAB{g}")
                for g in range(2)]
        silu_sb = keep.tile([MB2, DH], F16)
        st_own = keep.tile([128, 4 * MB2], F16)
        stA = keep.tile([128, 4 * MB2], F16)
        stB = keep.tile([128, 4 * MB2], F16)

        with tc.tile_pool(name="psA", bufs=1, space="PSUM") as psA, \
                tc.tile_pool(name="psX", bufs=1, space="PSUM") as psX, \
                tc.tile_pool(name="psB", bufs=1, space="PSUM") as psB, \
                tc.tile_pool(name="psD", bufs=1, space="PSUM") as psD:
            # ---- phase A: own-2-sample ctx_sumT[h, s] (fills the PE
            # window before the first x piece lands) ----
            ps_ctx = psA.tile([128, HT * SPC], F32)
            for kc in range(KC):
                for hc in range(HT):
                    nc.tensor.matmul(
                        ps_ctx[:, SPC * hc:SPC * (hc + 1)],
                        eh_sb[:, kc * H + 128 * hc:kc * H + 128 * (hc + 1)],
                        cm8_sb[:, SPC * kc:SPC * (kc + 1)],
                        start=(kc == 0 and hc == 0),
                        stop=(kc == KC - 1),
                    )
            nc.vector.tensor_copy(cs16, ps_ctx)
            # cs pair-exchange: first on the TOPSP chain, hidden under x
            nc.gpsimd.dma_start(out=csp_d[:, :], in_=cs16)
            nc.gpsimd.collective_compute(
                "AllGather", mybir.AluOpType.bypass, replica_groups=RG2,
                ins=[csp_d[:, :].opt()], outs=[csg_d[:, :].opt()])
            # gathered cs -> interleaved [A0 A1 B0 B1] per hc, f16->f8
            cv = cs4.rearrange("p (hc w) -> p hc w", w=4)
            nc.gpsimd.dma_start(out=cv[:, :, 0:2], in_=csg_d[0:128, :])
            nc.gpsimd.dma_start(out=cv[:, :, 2:4], in_=csg_d[128:256, :])

            # ---- phase X: xsumT[h, j] in 2 h-chunks of CH=(24, 8)
            # h-tiles; x streams through xpool, 2.25MB per buffer ----
            # The small trailing chunk keeps the one EXPOSED AllGather
            # (chunk 1, after the x stream ends) at a 0.125MB payload.
            ps_xs = psX.tile([128, HT * M], F32)  # 4 banks
            KPK = (3, 9)      # k-tiles per x piece (both 18KB/part)
            for g in range(2):
                hw = 128 * CH[g]          # chunk h-columns
                base = 0 if g == 0 else KT * 128 * CH[0]
                h0 = 0 if g == 0 else CH[0]
                kpk = KPK[g]
                for kb in range((KT + kpk - 1) // kpk):
                    nk = min(kpk, KT - kpk * kb)
                    xt = xpool.tile([128, 9 * 1024], F16, tag="xkb")
                    nc.sync.dma_start(
                        out=xt[:, 0:nk * hw],
                        in_=xh_d[:, base + kpk * kb * hw:
                                 base + (kpk * kb + nk) * hw])
                    for kk in range(nk):
                        k = kpk * kb + kk
                        for hcl in range(CH[g]):
                            nc.tensor.matmul(
                                ps_xs[:, (h0 + hcl) * M:(h0 + hcl + 1) * M],
                                xt[:, kk * hw + 128 * hcl:
                                   kk * hw + 128 * (hcl + 1)],
                                mxr_sb[:, M * k:M * (k + 1)],
                                start=(k == 0 and hcl % 8 == 0),
                                stop=(k == KT - 1),
                            )
                cg0, cg1 = h0 * M, (h0 + CH[g]) * M
                nc.vector.tensor_copy(xs_c[:, cg0:cg1], ps_xs[:, cg0:cg1])
                nc.gpsimd.dma_start(out=xsp_d[g][:, :], in_=xs_c[:, cg0:cg1])
                nc.gpsimd.collective_compute(
                    "AllGather", mybir.AluOpType.bypass, replica_groups=RG2,
                    ins=[xsp_d[g][:, :].opt()], outs=[xsg_d[g][:, :].opt()])
                nc.gpsimd.dma_start(out=xsA[g], in_=xsg_d[g][0:128, :])
                nc.gpsimd.dma_start(out=xsB[g], in_=xsg_d[g][128:256, :])

            # weights: behind x on the sync queue, ahead of D/B/U use
            nc.sync.dma_start(out=wch_sb, in_=wch_d[:, :])
            for q in range(2):
                c0, c1 = q * 16 * DH, (q + 1) * 16 * DH
                nc.sync.dma_start(out=wdh_sb[:, c0:c1], in_=wdh_d[:, c0:c1])
            for q in range(2):
                c0, c1 = q * 4 * HH, (q + 1) * 4 * HH
                nc.sync.dma_start(out=wuh_sb[:, c0:c1], in_=wuh_d[:, c0:c1])
            nc.sync.dma_start(out=buh_sb, in_=bu_d[:, :])

            # interleave slabs AFTER both xs copies on the DVE queue (a
            # copy waiting on chunk-0's AllGather must not head-of-line
            # block the chunk-1 xs copy): [128, hc x (A 64 | B 64)]
            for g in range(2):
                v = xsAB[g].rearrange("p (hc w) -> p hc w", w=128)
                nc.vector.tensor_copy(
                    v[:, :, 0:64],
                    xsA[g].rearrange("p (hc w) -> p hc w", w=64))
                nc.vector.tensor_copy(
                    v[:, :, 64:128],
                    xsB[g].rearrange("p (hc w) -> p hc w", w=64))

            # ---- phase D (chunk 0), B, D (chunk 1) ----
            ps_d = psD.tile([128, DH], F32)   # 1 bank
            ps_cd = psB.tile([4, DH], F32)    # 1 bank
            for hc in range(CH[0]):
                nc.tensor.matmul(
                    ps_d,
                    xsAB[0][:, 128 * hc:128 * (hc + 1)],
                    wdh_sb[:, DH * hc:DH * (hc + 1)],
                    start=(hc == 0), stop=False)
            for hc in range(HT):
                nc.tensor.matmul(
                    ps_cd,
                    cs4[:, 4 * hc:4 * (hc + 1)],
                    wch_sb[:, DH * hc:DH * (hc + 1)],
                    start=(hc == 0), stop=(hc == HT - 1))
            for hcl in range(CH[1]):
                hc = CH[0] + hcl
                nc.tensor.matmul(
                    ps_d,
                    xsAB[1][:, 128 * hcl:128 * (hcl + 1)],
                    wdh_sb[:, DH * hc:DH * (hc + 1)],
                    start=False, stop=False)
            nc.vector.tensor_copy(augr_sb[0:4, :], ps_cd)

            # aug term closes the D accumulation, then scaled silu
            nc.tensor.matmul(ps_d, aug_sb, augr_sb, start=False, stop=True)
            nc.scalar.activation(
                silu_sb, ps_d,
                mybir.ActivationFunctionType.Silu, scale=sinv_sb)

        # ---- phase E: siluT [d-half, 128 bins] ----
        with tc.tile_pool(name="psE", bufs=2, space="PSUM") as psE:
            for dc in range(4):
                pst = psE.tile([128, 128], F16, tag="silutr")
                nc.tensor.transpose(
                    pst, silu_sb[:, 128 * dc:128 * (dc + 1)], ident16)
                nc.vector.tensor_copy(
                    st_own[:, 128 * dc:128 * (dc + 1)], pst)
        nc.gpsimd.dma_start(out=stp_d[:, :], in_=st_own)
        nc.gpsimd.collective_compute(
            "AllGather", mybir.AluOpType.bypass, replica_groups=RG2,
            ins=[stp_d[:, :].opt()], outs=[stg_d[:, :].opt()])
        nc.gpsimd.dma_start(out=stA, in_=stg_d[0:128, :])
        nc.gpsimd.dma_start(out=stB, in_=stg_d[128:256, :])

        # ---- phase U: out[pair bins, h-half] ----
        with tc.tile_pool(name="psU", bufs=1, space="PSUM") as psU:
            ps_u = psU.tile([MB2, HH], F32)  # 4 banks
            for dk in range(DT):
                sl = stA if dk < 4 else stB
                lhsT = sl[:, 128 * (dk % 4):128 * (dk % 4 + 1)]
                for nb in range(4):
                    nc.tensor.matmul(
                        ps_u[:, 512 * nb:512 * (nb + 1)],
                        lhsT,
                        wuh_sb[:, HH * dk + 512 * nb:HH * dk + 512 * (nb + 1)],
                        start=(dk == 0), stop=False)
            for nb in range(4):
                nc.tensor.matmul(
                    ps_u[:, 512 * nb:512 * (nb + 1)],
                    ones1,
                    buh_sb[:, 512 * nb:512 * (nb + 1)],
                    start=False, stop=True)
            ot = opool.tile([MB2, HH], F16)
            for nb in range(4):
                nc.vector.tensor_copy(
                    ot[:, 512 * nb:512 * (nb + 1)],
                    ps_u[:, 512 * nb:512 * (nb + 1)])
                if nb % 2 == 1:
                    nc.sync.dma_start(
                        out=out_d[:, 512 * (nb - 1):512 * (nb + 1)],
                        in_=ot[:, 512 * (nb - 1):512 * (nb + 1)])

    nc.finalize()
    return nc


def _roundup(v, m):
    return max(m, ((int(v) + m - 1) // m) * m)


def _pm(a, kt):
    """Reorder [kt*128, cols] row-major -> partition-major [128, kt*cols]."""
    n, cols = a.shape
    assert n == kt * 128
    return np.ascontiguousarray(
        a.reshape(kt, 128, cols).transpose(1, 0, 2).reshape(128, kt * cols))


def _f8(a):
    return np.ascontiguousarray(np.asarray(a, dtype=np.float32)).astype(F8NP)


def _pair_samples(seq):
    """Pair the 16 samples 2-per-core minimizing roundup128(max pair seq).
    Greedy sort-and-reflect, then 2-opt passes."""
    order = np.argsort(-seq, kind="stable")
    pairs = [[int(order[i]), int(order[2 * NC - 1 - i])] for i in range(NC)]

    def cost(ps):
        return (_roundup(max(seq[a] + seq[b] for a, b in ps), 128),
                max(seq[a] + seq[b] for a, b in ps))

    best = cost(pairs)
    improved = True
    while improved:
        improved = False
        for i in range(NC):
            for j in range(i + 1, NC):
                for swap in ((1, 1), (1, 0), (0, 1)):
                    cand = [list(p) for p in pairs]
                    cand[i][swap[0]], cand[j][swap[1]] = \
                        cand[j][swap[1]], cand[i][swap[0]]
                    c = cost(cand)
                    if c < best:
                        best, pairs, improved = c, cand, True
    return [(a, b) for a, b in pairs]


def kernel(**inputs):
    ids = np.asarray(inputs["context_ids"]).astype(np.int64)
    x = np.asarray(inputs["hidden_states"], dtype=np.float32)
    seq = np.asarray(inputs["seq_lengths"]).astype(np.int64)
    clen = np.asarray(inputs["context_lengths"]).astype(np.int64)
    emb = np.asarray(inputs["embed_table"], dtype=np.float32)
    Wc = np.ascontiguousarray(inputs["Wc"], dtype=np.float32)
    bc = np.asarray(inputs["bc"], dtype=np.float32)
    Wd = np.ascontiguousarray(inputs["Wd"], dtype=np.float32)
    bd = np.asarray(inputs["bd"], dtype=np.float32)
    Wu = np.ascontiguousarray(inputs["Wu"], dtype=np.float32)
    bu = np.asarray(inputs["bu"], dtype=np.float32)

    assert x.shape == (B, S, H) and ids.shape == (B, C)

    # per-sample bin geometry
    L = seq + 1
    jj = np.arange(P, dtype=np.int64)
    start = (jj[None, :] * L[:, None]) // P            # [B,P]
    end = ((jj[None, :] + 1) * L[:, None] + P - 1) // P
    Sj = (end - start).astype(np.float32)
    lo = np.maximum(start - 1, 0)
    hi = end - 1
    cnt = (hi - lo).astype(np.float32)
    ind = (start == 0).astype(np.float32)

    pairs = _pair_samples(seq)
    T = _roundup(max(seq[a] + seq[b] for a, b in pairs), 128)
    KT = T // 128

    clen_c = np.maximum(clen, 1)
    Tc = _roundup(max(clen_c[a] + clen_c[b] for a, b in pairs), 128)
    KC = Tc // 128

    key = (T, Tc)
    if key not in _cache:
        _cache[key] = _build(T, Tc)
    nc = _cache[key]

    # host-side weight layouts
    emb8 = _f8(emb * EH_SCALE)
    wd16 = Wd.astype(np.float16)
    wu16 = Wu.astype(np.float16)
    wdh = [_pm(np.ascontiguousarray(wd16[:, DH * m:DH * (m + 1)]), HT)
           for m in range(2)]
    wch = [_pm(_f8(Wc[:, DH * m:DH * (m + 1)] * WC_SCALE), HT)
           for m in range(2)]
    wuh = [np.ascontiguousarray(
        wu16[:, HH * m:HH * (m + 1)].reshape(DT, 128, HH)
        .transpose(1, 0, 2).reshape(128, DT * HH)) for m in range(2)]
    bdh = [bd[DH * m:DH * (m + 1)].reshape(1, DH) for m in range(2)]
    bch = [bc[DH * m:DH * (m + 1)].reshape(1, DH) for m in range(2)]
    buh = [bu[HH * m:HH * (m + 1)].astype(np.float16).reshape(1, HH)
           for m in range(2)]

    in_maps = []
    for c, (a, b) in enumerate(pairs):
        g, m = c // 2, c % 2
        sa, sb = int(seq[a]), int(seq[b])
        xp = np.zeros((T, H), np.float16)
        xp[:sa] = x[a, :sa]
        xp[sa:sa + sb] = x[b, :sb]
        # chunk-major reorder: [128, KT x 3072 | KT x 1024] (24/8 h-tiles)
        xk = xp.reshape(KT, 128, H)
        xr = np.concatenate([
            np.ascontiguousarray(xk[:, :, 0:3072].transpose(1, 0, 2))
            .reshape(128, KT * 3072),
            np.ascontiguousarray(xk[:, :, 3072:H].transpose(1, 0, 2))
            .reshape(128, KT * 1024),
        ], axis=1)
        t = np.arange(T, dtype=np.int64)[:, None]
        mx = np.zeros((T, M), np.float16)
        mx[:, :P] = ((t >= lo[a][None, :]) & (t < hi[a][None, :]))
        mx[:, P:] = ((t - sa >= lo[b][None, :]) & (t - sa < hi[b][None, :])
                     & (t >= sa))
        # own 2 samples' context rows + 2-col one-hot
        ca, cb = int(clen_c[a]), int(clen_c[b])
        ep = np.zeros((Tc, H), F8NP)
        ep[:ca] = emb8[ids[a, :ca]]
        ep[ca:ca + cb] = emb8[ids[b, :cb]]
        cm = np.zeros((Tc, SPC), np.float32)
        cm[:ca, 0] = 1.0
        cm[ca:ca + cb, 1] = 1.0
        # group bin order: [pair0 sample a bins, pair0 b, pair1 a, pair1 b]
        gs = [pairs[2 * g][0], pairs[2 * g][1],
              pairs[2 * g + 1][0], pairs[2 * g + 1][1]]
        # augr rows: 0..3 = ctx_d of gs[0..3]; 4 = bd; 5 = bc
        aug = np.zeros((6, MB2), np.float32)
        sinv = np.zeros((MB2, 1), np.float32)
        for i, s in enumerate(gs):
            sl = slice(P * i, P * (i + 1))
            aug[i, sl] = ind[s] / (clen_c[s] * CTX_SCALE)
            aug[4, sl] = cnt[s]
            aug[5, sl] = ind[s]
            sinv[sl, 0] = 1.0 / Sj[s]
        in_maps.append({
            "xh": xr, "mxr": _pm(mx, KT),
            "eh8": _pm(ep, KC), "cm8": _pm(cm.astype(F8NP), KC),
            "wch8": wch[m], "wdh": wdh[m], "wuh": wuh[m],
            "bdh": bdh[m], "bch": bch[m], "buh": buh[m],
            "aug": aug, "sinv": sinv,
        })

    res = run_bass_kernel_spmd(nc, in_maps, core_ids=list(range(NC)))
    _cache["last_result"] = res

    out = np.empty((B, P, H), np.float32)
    for c in range(NC):
        g, m = c // 2, c % 2
        o = np.asarray(res.results[c]["out"], dtype=np.float32)
        gs = [pairs[2 * g][0], pairs[2 * g][1],
              pairs[2 * g + 1][0], pairs[2 * g + 1][1]]
        for i, s in enumerate(gs):
            out[s, :, HH * m:HH * (m + 1)] = o[P * i:P * (i + 1), :]
    return out


# revision 22
# speedup vs baseline: 1.1876x; 1.0865x over previous
"""Trainium2 Bass kernel for nn_ContextPromptGenerator.

Math restructure (as baseline): pooled bins are masked segment sums over
tokens, so the 0/1-mask matmul runs FIRST on [T, 4096] packed rows, then
the 4096->1024 down-projection runs on pooled rows only.

v3 over the 201us baseline:
- TP-pair: cores are paired {2g, 2g+1}; a pair shares its 4 samples
  (128 bins).  Each member computes ALL 128 pair bins but only HALF of
  the D dimension (Wd[:, 512m:+512], 4MB) and HALF of the H output
  (Wu[:, 2048m:+2048], 4MB) -- weight DMA drops 16MB -> 8MB/core.  Bin
  sums are exchanged via 2-rank AllGathers (2 x 256KB xsT chunks,
  1 x 128KB siluT) on replica groups [[0,1],[2,3],[4,5],[6,7]].
- ctx path is pair-local: each core sums ONLY its own 2 samples'
  context embeddings (no cross-core reduction exists, so the 42us
  8-rank AllReduce of v2 is gone); a 16KB pair-AllGather supplies the
  partner's sums; ctx_d[4, 512-half] comes from the fp8 Wc half.
- eh/cmask/Wc-half are fp8e4 (emb x64, Wc x16; compensated in the fp32
  aug coefficients /1024).  Verified: error unchanged (4.1e-4).
- Phase D runs full-array: the two xs slabs are interleaved on-chip
  (DVE strided copy) into [128, hc x 128bins] so each D matmul carries
  128 weight columns.
- DMA: one sync-queue FIFO in priority order (masks/eh -> x -> Wd ->
  Wc -> Wu); x streams through a 3-buffer pool in 1MB pieces; weights
  are 0.5-4MB pieces consumed slice-wise by D/U as they land.
- Output is written fp16 (host upcasts); halves the out DMA.

Sharding: data-parallel over samples for the bin sums (2 per core,
paired to minimize roundup128(max seq pair)); TP-pair for D/U and ctx.
"""

import numpy as np
import ml_dtypes
from contextlib import ExitStack

import concourse.bass as bass
import concourse.mybir as mybir
import concourse.tile as tile
from concourse import bacc
from concourse.masks import make_identity
from concourse.bass_utils import run_bass_kernel_spmd

F32 = mybir.dt.float32
F16 = mybir.dt.float16
F8 = mybir.dt.float8e4
F8NP = ml_dtypes.float8_e4m3

B, S, C, H, D, V, P = 16, 2048, 512, 4096, 1024, 32000, 32
NC = 8          # cores
SPC = 2         # samples per core
M = SPC * P     # 64 own bins per core
MB2 = 2 * M     # 128 bins per core-pair
HT = H // 128   # 32 h-tiles
DT = D // 128   # 8 d-tiles
DH = D // 2     # 512: d half (per pair member)
HH = H // 2     # 2048: h half (per pair member)
RG2 = [[0, 1], [2, 3], [4, 5], [6, 7]]
EH_SCALE = 64.0      # emb fp8 pre-scale
WC_SCALE = 16.0      # Wc fp8 pre-scale
CTX_SCALE = EH_SCALE * WC_SCALE

_cache = {}


def _build(T, Tc):
    """Per-core SPMD Bass program.  T = packed hidden rows per core,
    Tc = packed own-2-sample context rows per core (both mult. of 128)."""
    nc = bacc.Bacc(None, target_bir_lowering=False, num_devices=NC)

    KT = T // 128
    KC = Tc // 128

    # ---- dram I/O ----
    xh_d = nc.dram_tensor("xh", [128, KT * H], F16, kind="ExternalInput")
    mx_d = nc.dram_tensor("mxr", [128, KT * M], F16, kind="ExternalInput")
    eh_d = nc.dram_tensor("eh8", [128, KC * H], F8, kind="ExternalInput")
    cm_d = nc.dram_tensor("cm8", [128, KC * SPC], F8, kind="ExternalInput")
    wch_d = nc.dram_tensor("wch8", [128, HT * DH], F8, kind="ExternalInput")
    wdh_d = nc.dram_tensor("wdh", [128, HT * DH], F16, kind="ExternalInput")
    wuh_d = nc.dram_tensor("wuh", [128, DT * HH], F16, kind="ExternalInput")
    bd_d = nc.dram_tensor("bdh", [1, DH], F32, kind="ExternalInput")
    bc_d = nc.dram_tensor("bch", [1, DH], F32, kind="ExternalInput")
    bu_d = nc.dram_tensor("buh", [1, HH], F16, kind="ExternalInput")
    aug_d = nc.dram_tensor("aug", [36, MB2], F32, kind="ExternalInput")
    sinv_d = nc.dram_tensor("sinv", [MB2, 1], F32, kind="ExternalInput")
    out_d = nc.dram_tensor("out", [MB2, HH], F16, kind="ExternalOutput")
    # collective bounce buffers (internal DRAM)
    csp_d = nc.dram_tensor("csp", [128, HT * SPC], F16)
    csg_d = nc.dram_tensor("csg", [256, HT * SPC], F16)
    CH = (24, 8)  # h-tiles per X chunk (bank-aligned: 3 banks + 1)
    xsp_d = [nc.dram_tensor(f"xsp{g}", [128, CH[g] * M], F16)
             for g in range(2)]
    xsg_d = [nc.dram_tensor(f"xsg{g}", [256, CH[g] * M], F16)
             for g in range(2)]
    stp_d = nc.dram_tensor("stp", [128, 4 * MB2], F16)
    stg_d = nc.dram_tensor("stg", [256, 4 * MB2], F16)

    with tile.TileContext(nc) as tc, ExitStack() as ctx:
        const = ctx.enter_context(tc.tile_pool(name="const", bufs=1))
        keep = ctx.enter_context(tc.tile_pool(name="keep", bufs=1))
        xpool = ctx.enter_context(tc.tile_pool(name="xpool", bufs=3))
        opool = ctx.enter_context(tc.tile_pool(name="opool", bufs=1))

        ident16 = const.tile([128, 128], F16)
        idtmp = const.tile([128, 128], F32)
        make_identity(nc, idtmp)
        nc.vector.tensor_copy(ident16, idtmp)
        ones1 = const.tile([1, 128], F16)
        nc.vector.memset(ones1, 1.0)

        # ---- sync-queue input DMAs, emitted in priority order ----
        cm8_sb = keep.tile([128, KC * SPC], F8)
        nc.sync.dma_start(out=cm8_sb, in_=cm_d[:, :])
        eh_sb = keep.tile([128, KC * H], F8)
        ehmid = (KC // 2) * H
        if ehmid > 0:
            nc.sync.dma_start(out=eh_sb[:, 0:ehmid], in_=eh_d[:, 0:ehmid])
        nc.sync.dma_start(out=eh_sb[:, ehmid:], in_=eh_d[:, ehmid:])
        mxr_sb = keep.tile([128, KT * M], F16)
        nc.sync.dma_start(out=mxr_sb, in_=mx_d[:, :])
        aug_sb = keep.tile([36, MB2], F32)
        nc.sync.dma_start(out=aug_sb, in_=aug_d[:, :])
        sinv_sb = keep.tile([MB2, 1], F32)
        nc.sync.dma_start(out=sinv_sb, in_=sinv_d[:, :])
        augr_sb = keep.tile([36, DH], F32)
        nc.vector.memset(augr_sb, 0.0)
        nc.sync.dma_start(out=augr_sb[34:35, :], in_=bd_d[:, :])
        nc.sync.dma_start(out=augr_sb[35:36, :], in_=bc_d[:, :])

        # weight tiles (DMAs emitted AFTER phase X: x streams first)
        wdh_sb = keep.tile([128, HT * DH], F16)
        wch_sb = keep.tile([128, HT * DH], F8)
        wuh_sb = keep.tile([128, DT * HH], F16)
        buh_sb = keep.tile([1, HH], F16)

        cs16 = keep.tile([128, HT * SPC], F16)
        csA8 = keep.tile([128, HT * SPC], F8)
        csB8 = keep.tile([128, HT * SPC], F8)
        xs_c = keep.tile([128, HT * M], F16)
        xsA = [keep.tile([128, CH[g] * M], F16, name=f"xsA{g}")
               for g in range(2)]
        xsB = [keep.tile([128, CH[g] * M], F16, name=f"xsB{g}")
               for g in range(2)]
        xsAB = [keep.tile([128, CH[g] * 128], F16, name=f"xsAB{g}")
                for g in range(2)]
        silu_sb = keep.tile([MB2, DH], F16)
        st_own = keep.tile([128, 4 * MB2], F16)
        stA = keep.tile([128, 4 * MB2], F16)
        stB = keep.tile([128, 4 * MB2], F16)

        with tc.tile_pool(name="psA", bufs=1, space="PSUM") as psA, \
                tc.tile_pool(name="psX", bufs=1, space="PSUM") as psX, \
                tc.tile_pool(name="psB", bufs=1, space="PSUM") as psB, \
                tc.tile_pool(name="psD", bufs=1, space="PSUM") as psD:
            # ---- phase A: own-2-sample ctx_sumT[h, s] (fills the PE
            # window before the first x piece lands) ----
            ps_ctx = psA.tile([128, HT * SPC], F32)
            for kc in range(KC):
                for hc in range(HT):
                    nc.tensor.matmul(
                        ps_ctx[:, SPC * hc:SPC * (hc + 1)],
                        eh_sb[:, kc * H + 128 * hc:kc * H + 128 * (hc + 1)],
                        cm8_sb[:, SPC * kc:SPC * (kc + 1)],
                        start=(kc == 0 and hc == 0),
                        stop=(kc == KC - 1),
                    )
            nc.vector.tensor_copy(cs16, ps_ctx)
            # cs pair-exchange: first on the TOPSP chain, hidden under x
            nc.gpsimd.dma_start(out=csp_d[:, :], in_=cs16)
            nc.gpsimd.collective_compute(
                "AllGather", mybir.AluOpType.bypass, replica_groups=RG2,
                ins=[csp_d[:, :].opt()], outs=[csg_d[:, :].opt()])
            nc.gpsimd.dma_start(out=csA8, in_=csg_d[0:128, :])    # f16->f8
            nc.gpsimd.dma_start(out=csB8, in_=csg_d[128:256, :])  # f16->f8

            # wch ahead of x: phase B's matmuls are ready at X-end and
            # fill the PE window while the wd stream lands
            nc.sync.dma_start(out=wch_sb, in_=wch_d[:, :])

            # ---- phase X: xsumT[h, j] in 2 h-chunks of CH=(24, 8)
            # h-tiles; x streams through xpool, 2.25MB per buffer ----
            # The small trailing chunk keeps the one EXPOSED AllGather
            # (chunk 1, after the x stream ends) at a 0.125MB payload.
            ps_xs = psX.tile([128, HT * M], F32)  # 4 banks
            KPK = (3, 9)      # k-tiles per x piece (both 18KB/part)
            for g in range(2):
                hw = 128 * CH[g]          # chunk h-columns
                base = 0 if g == 0 else KT * 128 * CH[0]
                h0 = 0 if g == 0 else CH[0]
                kpk = KPK[g]
                for kb in range((KT + kpk - 1) // kpk):
                    nk = min(kpk, KT - kpk * kb)
                    xt = xpool.tile([128, 9 * 1024], F16, tag="xkb")
                    nc.sync.dma_start(
                        out=xt[:, 0:nk * hw],
                        in_=xh_d[:, base + kpk * kb * hw:
                                 base + (kpk * kb + nk) * hw])
                    for kk in range(nk):
                        k = kpk * kb + kk
                        for hcl in range(CH[g]):
                            nc.tensor.matmul(
                                ps_xs[:, (h0 + hcl) * M:(h0 + hcl + 1) * M],
                                xt[:, kk * hw + 128 * hcl:
                                   kk * hw + 128 * (hcl + 1)],
                                mxr_sb[:, M * k:M * (k + 1)],
                                start=(k == 0 and hcl % 8 == 0),
                                stop=(k == KT - 1),
                            )
                cg0, cg1 = h0 * M, (h0 + CH[g]) * M
                nc.vector.tensor_copy(xs_c[:, cg0:cg1], ps_xs[:, cg0:cg1])
                nc.gpsimd.dma_start(out=xsp_d[g][:, :], in_=xs_c[:, cg0:cg1])
                nc.gpsimd.collective_compute(
                    "AllGather", mybir.AluOpType.bypass, replica_groups=RG2,
                    ins=[xsp_d[g][:, :].opt()], outs=[xsg_d[g][:, :].opt()])
                nc.gpsimd.dma_start(out=xsA[g], in_=xsg_d[g][0:128, :])
                nc.gpsimd.dma_start(out=xsB[g], in_=xsg_d[g][128:256, :])

            # weights: behind x on the sync queue, ahead of D/U use
            for q in range(2):
                c0, c1 = q * 16 * DH, (q + 1) * 16 * DH
                nc.sync.dma_start(out=wdh_sb[:, c0:c1], in_=wdh_d[:, c0:c1])
            for q in range(2):
                c0, c1 = q * 4 * HH, (q + 1) * 4 * HH
                nc.sync.dma_start(out=wuh_sb[:, c0:c1], in_=wuh_d[:, c0:c1])
            nc.sync.dma_start(out=buh_sb, in_=bu_d[:, :])

            # interleave slabs AFTER both xs copies on the DVE queue (a
            # copy waiting on chunk-0's AllGather must not head-of-line
            # block the chunk-1 xs copy): [128, hc x (A 64 | B 64)]
            for g in range(2):
                v = xsAB[g].rearrange("p (hc w) -> p hc w", w=128)
                nc.vector.tensor_copy(
                    v[:, :, 0:64],
                    xsA[g].rearrange("p (hc w) -> p hc w", w=64))
                nc.vector.tensor_copy(
                    v[:, :, 64:128],
                    xsB[g].rearrange("p (hc w) -> p hc w", w=64))

            # ---- phase B (2 chains, ready at X-end), then D ----
            ps_d = psD.tile([128, DH], F32)   # 1 bank
            ps_cdA = psB.tile([2, DH], F32)   # 1 bank
            ps_cdB = psB.tile([2, DH], F32)   # 1 bank
            for hc in range(HT):
                nc.tensor.matmul(
                    ps_cdA,
                    csA8[:, SPC * hc:SPC * (hc + 1)],
                    wch_sb[:, DH * hc:DH * (hc + 1)],
                    start=(hc == 0), stop=(hc == HT - 1))
            for hc in range(HT):
                nc.tensor.matmul(
                    ps_cdB,
                    csB8[:, SPC * hc:SPC * (hc + 1)],
                    wch_sb[:, DH * hc:DH * (hc + 1)],
                    start=(hc == 0), stop=(hc == HT - 1))
            nc.vector.tensor_copy(augr_sb[0:2, :], ps_cdA)
            nc.vector.tensor_copy(augr_sb[32:34, :], ps_cdB)
            for hc in range(CH[0]):
                nc.tensor.matmul(
                    ps_d,
                    xsAB[0][:, 128 * hc:128 * (hc + 1)],
                    wdh_sb[:, DH * hc:DH * (hc + 1)],
                    start=(hc == 0), stop=False)
            for hcl in range(CH[1]):
                hc = CH[0] + hcl
                nc.tensor.matmul(
                    ps_d,
                    xsAB[1][:, 128 * hcl:128 * (hcl + 1)],
                    wdh_sb[:, DH * hc:DH * (hc + 1)],
                    start=False, stop=False)

            # aug term closes the D accumulation, then scaled silu
            nc.tensor.matmul(ps_d, aug_sb, augr_sb, start=False, stop=True)
            nc.scalar.activation(
                silu_sb, ps_d,
                mybir.ActivationFunctionType.Silu, scale=sinv_sb)

        # ---- phase E: siluT [d-half, 128 bins] ----
        with tc.tile_pool(name="psE", bufs=2, space="PSUM") as psE:
            for dc in range(4):
                pst = psE.tile([128, 128], F16, tag="silutr")
                nc.tensor.transpose(
                    pst, silu_sb[:, 128 * dc:128 * (dc + 1)], ident16)
                nc.vector.tensor_copy(
                    st_own[:, 128 * dc:128 * (dc + 1)], pst)
        nc.gpsimd.dma_start(out=stp_d[:, :], in_=st_own)
        nc.gpsimd.collective_compute(
            "AllGather", mybir.AluOpType.bypass, replica_groups=RG2,
            ins=[stp_d[:, :].opt()], outs=[stg_d[:, :].opt()])
        nc.gpsimd.dma_start(out=stA, in_=stg_d[0:128, :])
        nc.gpsimd.dma_start(out=stB, in_=stg_d[128:256, :])

        # ---- phase U: out[pair bins, h-half] ----
        with tc.tile_pool(name="psU", bufs=1, space="PSUM") as psU:
            ps_u = psU.tile([MB2, HH], F32)  # 4 banks
            for dk in range(DT):
                sl = stA if dk < 4 else stB
                lhsT = sl[:, 128 * (dk % 4):128 * (dk % 4 + 1)]
                for nb in range(4):
                    nc.tensor.matmul(
                        ps_u[:, 512 * nb:512 * (nb + 1)],
                        lhsT,
                        wuh_sb[:, HH * dk + 512 * nb:HH * dk + 512 * (nb + 1)],
                        start=(dk == 0), stop=False)
            for nb in range(4):
                nc.tensor.matmul(
                    ps_u[:, 512 * nb:512 * (nb + 1)],
                    ones1,
                    buh_sb[:, 512 * nb:512 * (nb + 1)],
                    start=False, stop=True)
            ot = opool.tile([MB2, HH], F16)
            for nb in range(4):
                nc.vector.tensor_copy(
                    ot[:, 512 * nb:512 * (nb + 1)],
                    ps_u[:, 512 * nb:512 * (nb + 1)])
                if nb % 2 == 1:
                    nc.sync.dma_start(
                        out=out_d[:, 512 * (nb - 1):512 * (nb + 1)],
                        in_=ot[:, 512 * (nb - 1):512 * (nb + 1)])

    nc.finalize()
    return nc


def _roundup(v, m):
    return max(m, ((int(v) + m - 1) // m) * m)


def _pm(a, kt):
    """Reorder [kt*128, cols] row-major -> partition-major [128, kt*cols]."""
    n, cols = a.shape
    assert n == kt * 128
    return np.ascontiguousarray(
        a.reshape(kt, 128, cols).transpose(1, 0, 2).reshape(128, kt * cols))


def _f8(a):
    return np.ascontiguousarray(np.asarray(a, dtype=np.float32)).astype(F8NP)


def _pair_samples(seq):
    """Pair the 16 samples 2-per-core minimizing roundup128(max pair seq).
    Greedy sort-and-reflect, then 2-opt passes."""
    order = np.argsort(-seq, kind="stable")
    pairs = [[int(order[i]), int(order[2 * NC - 1 - i])] for i in range(NC)]

    def cost(ps):
        return (_roundup(max(seq[a] + seq[b] for a, b in ps), 128),
                max(seq[a] + seq[b] for a, b in ps))

    best = cost(pairs)
    improved = True
    while improved:
        improved = False
        for i in range(NC):
            for j in range(i + 1, NC):
                for swap in ((1, 1), (1, 0), (0, 1)):
                    cand = [list(p) for p in pairs]
                    cand[i][swap[0]], cand[j][swap[1]] = \
                        cand[j][swap[1]], cand[i][swap[0]]
                    c = cost(cand)
                    if c < best:
                        best, pairs, improved = c, cand, True
    return [(a, b) for a, b in pairs]


def kernel(**inputs):
    ids = np.asarray(inputs["context_ids"]).astype(np.int64)
    x = np.asarray(inputs["hidden_states"], dtype=np.float32)
    seq = np.asarray(inputs["seq_lengths"]).astype(np.int64)
    clen = np.asarray(inputs["context_lengths"]).astype(np.int64)
    emb = np.asarray(inputs["embed_table"], dtype=np.float32)
    Wc = np.ascontiguousarray(inputs["Wc"], dtype=np.float32)
    bc = np.asarray(inputs["bc"], dtype=np.float32)
    Wd = np.ascontiguousarray(inputs["Wd"], dtype=np.float32)
    bd = np.asarray(inputs["bd"], dtype=np.float32)
    Wu = np.ascontiguousarray(inputs["Wu"], dtype=np.float32)
    bu = np.asarray(inputs["bu"], dtype=np.float32)

    assert x.shape == (B, S, H) and ids.shape == (B, C)

    # per-sample bin geometry
    L = seq + 1
    jj = np.arange(P, dtype=np.int64)
    start = (jj[None, :] * L[:, None]) // P            # [B,P]
    end = ((jj[None, :] + 1) * L[:, None] + P - 1) // P
    Sj = (end - start).astype(np.float32)
    lo = np.maximum(start - 1, 0)
    hi = end - 1
    cnt = (hi - lo).astype(np.float32)
    ind = (start == 0).astype(np.float32)

    pairs = _pair_samples(seq)
    T = _roundup(max(seq[a] + seq[b] for a, b in pairs), 128)
    KT = T // 128

    clen_c = np.maximum(clen, 1)
    Tc = _roundup(max(clen_c[a] + clen_c[b] for a, b in pairs), 128)
    KC = Tc // 128

    key = (T, Tc)
    if key not in _cache:
        _cache[key] = _build(T, Tc)
    nc = _cache[key]

    # host-side weight layouts
    emb8 = _f8(emb * EH_SCALE)
    wd16 = Wd.astype(np.float16)
    wu16 = Wu.astype(np.float16)
    wdh = [_pm(np.ascontiguousarray(wd16[:, DH * m:DH * (m + 1)]), HT)
           for m in range(2)]
    wch = [_pm(_f8(Wc[:, DH * m:DH * (m + 1)] * WC_SCALE), HT)
           for m in range(2)]
    wuh = [np.ascontiguousarray(
        wu16[:, HH * m:HH * (m + 1)].reshape(DT, 128, HH)
        .transpose(1, 0, 2).reshape(128, DT * HH)) for m in range(2)]
    bdh = [bd[DH * m:DH * (m + 1)].reshape(1, DH) for m in range(2)]
    bch = [bc[DH * m:DH * (m + 1)].reshape(1, DH) for m in range(2)]
    buh = [bu[HH * m:HH * (m + 1)].astype(np.float16).reshape(1, HH)
           for m in range(2)]

    in_maps = []
    for c, (a, b) in enumerate(pairs):
        g, m = c // 2, c % 2
        sa, sb = int(seq[a]), int(seq[b])
        xp = np.zeros((T, H), np.float16)
        xp[:sa] = x[a, :sa]
        xp[sa:sa + sb] = x[b, :sb]
        # chunk-major reorder: [128, KT x 3072 | KT x 1024] (24/8 h-tiles)
        xk = xp.reshape(KT, 128, H)
        xr = np.concatenate([
            np.ascontiguousarray(xk[:, :, 0:3072].transpose(1, 0, 2))
            .reshape(128, KT * 3072),
            np.ascontiguousarray(xk[:, :, 3072:H].transpose(1, 0, 2))
            .reshape(128, KT * 1024),
        ], axis=1)
        t = np.arange(T, dtype=np.int64)[:, None]
        mx = np.zeros((T, M), np.float16)
        mx[:, :P] = ((t >= lo[a][None, :]) & (t < hi[a][None, :]))
        mx[:, P:] = ((t - sa >= lo[b][None, :]) & (t - sa < hi[b][None, :])
                     & (t >= sa))
        # own 2 samples' context rows + 2-col one-hot
        ca, cb = int(clen_c[a]), int(clen_c[b])
        ep = np.zeros((Tc, H), F8NP)
        ep[:ca] = emb8[ids[a, :ca]]
        ep[ca:ca + cb] = emb8[ids[b, :cb]]
        cm = np.zeros((Tc, SPC), np.float32)
        cm[:ca, 0] = 1.0
        cm[ca:ca + cb, 1] = 1.0
        # group bin order: [pair0 sample a bins, pair0 b, pair1 a, pair1 b]
        gs = [pairs[2 * g][0], pairs[2 * g][1],
              pairs[2 * g + 1][0], pairs[2 * g + 1][1]]
        # augr rows: 0,1 = member0 ctx_d; 32,33 = member1 (32-aligned
        # partition bases for DVE copies); 34 = bd; 35 = bc
        aug = np.zeros((36, MB2), np.float32)
        sinv = np.zeros((MB2, 1), np.float32)
        for i, s in enumerate(gs):
            sl = slice(P * i, P * (i + 1))
            aug[i if i < 2 else 30 + i, sl] = ind[s] / (clen_c[s] * CTX_SCALE)
            aug[34, sl] = cnt[s]
            aug[35, sl] = ind[s]
            sinv[sl, 0] = 1.0 / Sj[s]
        in_maps.append({
            "xh": xr, "mxr": _pm(mx, KT),
            "eh8": _pm(ep, KC), "cm8": _pm(cm.astype(F8NP), KC),
            "wch8": wch[m], "wdh": wdh[m], "wuh": wuh[m],
            "bdh": bdh[m], "bch": bch[m], "buh": buh[m],
            "aug": aug, "sinv": sinv,
        })

    res = run_bass_kernel_spmd(nc, in_maps, core_ids=list(range(NC)))
    _cache["last_result"] = res

    out = np.empty((B, P, H), np.float32)
    for c in range(NC):
        g, m = c // 2, c % 2
        o = np.asarray(res.results[c]["out"], dtype=np.float32)
        gs = [pairs[2 * g][0], pairs[2 * g][1],
              pairs[2 * g + 1][0], pairs[2 * g + 1][1]]
        for i, s in enumerate(gs):
            out[s, :, HH * m:HH * (m + 1)] = o[P * i:P * (i + 1), :]
    return out
